# revision 14
# baseline (speedup 1.0000x reference)
"""Trainium2 Bass kernel for DAC-style residual VQ bottleneck (9 stages).

Algorithm restructure (validated vs reference to ~1e-7, 0 code mismatches):
  - Residual chain collapsed: z_e_i = InW_i @ x - sum_{j<i} (InW_i@OutW_j) @ zq_j.
    The K=1024 projections happen once ("base", stacked) and per-stage work is
    tiny K<=128 matmuls against a workspace whose rows go base_j -> zq_j.
  - Workspace is quadrant-padded: stage j<8 lives at partition 32*(j%4) of
    block j//4 so every engine access is quadrant-aligned (HW requirement);
    stage 8 uses a separate [8, .] path. Single writer engine (ACT) keeps
    per-instruction sync-wait counts low (HW limit).
  - argmin(dist) == argmax(enc . cb_normalized); top-1 via DVE max8+max_index,
    codebook row gather via indirect DMA, transposed back on the PE.
  - z = OutW_stacked @ zq_all; commit_loss == cb_loss partials shipped to host.

Sharding: one batch per NeuronCore (B=8), weights replicated.
"""

import numpy as np

B, D, T = 8, 1024, 2048
NQ, CS, CD = 9, 1024, 8
NCORES = 8
NCHUNK = 4          # column chunks per core
NC_COLS = 512       # columns per chunk
NST = 4             # 128-row subtiles per chunk
NDT = 8             # D tiles of 128
KW = NQ * CD        # 72 stacked rows
NBLK = 2            # quadrant-padded blocks (stages 0..7); stage 8 separate

_CACHE = {}


def _build_program(with_bias):
    import concourse.bass as bass
    import concourse.mybir as mybir
    from concourse import bacc
    from concourse.tile import TileContext
    from concourse.masks import make_identity

    f32 = mybir.dt.float32
    u32 = mybir.dt.uint32

    nc = bacc.Bacc()

    # ---- DRAM I/O ----
    x_d = nc.dram_tensor("x", [D, T], f32, kind="ExternalInput")
    wtin3_d = nc.dram_tensor("wtin3", [D, NBLK, 128], f32, kind="ExternalInput")
    wtin8_d = nc.dram_tensor("wtin8", [D, CD], f32, kind="ExternalInput")
    gal3_d = nc.dram_tensor("gal3", [128, NBLK, KW], f32, kind="ExternalInput")
    cbnt_d = nc.dram_tensor("cbnt", [CD, NQ * CS], f32, kind="ExternalInput")
    outwt3_d = nc.dram_tensor("outwt3", [128, NBLK, D], f32,
                              kind="ExternalInput")
    outwt8_d = nc.dram_tensor("outwt8", [CD, D], f32, kind="ExternalInput")
    if with_bias:
        cbias_d = nc.dram_tensor("cbias", [1, KW], f32, kind="ExternalInput")
        zbias_d = nc.dram_tensor("zbias", [1, D], f32, kind="ExternalInput")
    gtab_d = [
        nc.dram_tensor(f"gtab{i}", [CS, CD], f32, kind="ExternalInput")
        for i in range(NQ)
    ]
    z_d = nc.dram_tensor("z", [D, T], f32, kind="ExternalOutput")
    codes_d = nc.dram_tensor("codesst", [128, NQ * NCHUNK * NST * 8], u32,
                             kind="ExternalOutput")
    loss_d = nc.dram_tensor("lossp", [CD, 1], f32, kind="ExternalOutput")

    with TileContext(nc) as tc:
        with (
            tc.tile_pool(name="const", bufs=1) as constp,
            tc.tile_pool(name="xin", bufs=2) as xpool,
            tc.tile_pool(name="work", bufs=2) as wpool,
            tc.tile_pool(name="enc", bufs=3) as encp,
            tc.tile_pool(name="scores", bufs=6) as scp,
            tc.tile_pool(name="small", bufs=8) as smallp,
            tc.tile_pool(name="zout", bufs=3) as zp,
            tc.tile_pool(name="psA", bufs=1, space="PSUM") as ps_base,
            tc.tile_pool(name="psE", bufs=1, space="PSUM") as ps_enc,
            tc.tile_pool(name="psS", bufs=1, space="PSUM") as ps_sc,
            tc.tile_pool(name="psQ", bufs=1, space="PSUM") as ps_zq,
            tc.tile_pool(name="psQQ", bufs=1, space="PSUM") as ps_zqq,
            tc.tile_pool(name="psZ", bufs=1, space="PSUM") as ps_z,
        ):
            # ---- constants / weights to SBUF ----
            wtin3_t = constp.tile([128, NDT, NBLK, 128], f32, tag="wtin3")
            nc.sync.dma_start(
                out=wtin3_t[:],
                in_=wtin3_d.rearrange("(a p) b m -> p a b m", p=128))
            wtin8_t = constp.tile([128, NDT, CD], f32, tag="wtin8")
            nc.sync.dma_start(
                out=wtin8_t[:],
                in_=wtin8_d.rearrange("(a p) m -> p a m", p=128))
            gal3_t = constp.tile([128, NBLK, KW], f32, tag="gal3")
            nc.sync.dma_start(out=gal3_t[:], in_=gal3_d[:])
            cbnt_t = constp.tile([CD, NQ * CS], f32, tag="cbnt")
            nc.sync.dma_start(out=cbnt_t[:], in_=cbnt_d[:])
            outwt3_t = constp.tile([128, NBLK, D], f32, tag="outwt3")
            nc.sync.dma_start(out=outwt3_t[:], in_=outwt3_d[:])
            outwt8_t = constp.tile([CD, D], f32, tag="outwt8")
            nc.sync.dma_start(out=outwt8_t[:], in_=outwt8_d[:])
            if with_bias:
                cbias_t = constp.tile([1, KW], f32, tag="cbias")
                nc.sync.dma_start(out=cbias_t[:], in_=cbias_d[:])
                zbias_t = constp.tile([1, D], f32, tag="zbias")
                nc.sync.dma_start(out=zbias_t[:], in_=zbias_d[:])
                ones_t = constp.tile([1, NC_COLS], f32, tag="ones")
                nc.vector.memset(ones_t[:], 1.0)
            ident = constp.tile([128, 128], f32, tag="ident")
            make_identity(nc, ident[:])
            # PE warm-reads: absorb each weight tensor's DMA wait into the
            # PE vector clock up front (matmuls may carry only 1 sync wait).
            warm_ps = ps_base.tile([1, 16], f32, tag="base")
            warm_srcs = [wtin3_t[:, 0, 0, 0:1], wtin8_t[:, 0, 0:1],
                         gal3_t[:, 0, 0:1], cbnt_t[:, 0:1],
                         outwt3_t[:, 0, 0:1], outwt8_t[:, 0:1], ident[:, 0:1]]
            if with_bias:
                warm_srcs += [cbias_t[:, 0:1], zbias_t[:, 0:1], ones_t[:, 0:1]]
            for k, src in enumerate(warm_srcs):
                nc.tensor.matmul(warm_ps[0:1, k:k + 1], lhsT=src, rhs=src,
                                 start=True, stop=True)
            idxall = constp.tile([128, NQ * NCHUNK * NST * 8], u32, tag="idxall")
            loss_sb = constp.tile([CD, 40], f32, tag="loss")
            nc.vector.memset(loss_sb[:], 0)
            lossred = constp.tile([CD, 1], f32, tag="lossred")

            for c in range(NCHUNK):
                cols = slice(c * NC_COLS, (c + 1) * NC_COLS)
                # ---- load x chunk (single DMA -> single wait lane) ----
                xt = xpool.tile([128, NDT, NC_COLS], f32, tag="x")
                nc.sync.dma_start(
                    out=xt[:],
                    in_=x_d[:, cols].rearrange("(a p) n -> p a n", p=128))
                # ---- base (quadrant-padded blocks + stage-8) ----
                W3 = wpool.tile([128, NBLK, NC_COLS], f32, tag="W3")
                for bk in range(NBLK):
                    base_ps = ps_base.tile([128, NC_COLS], f32, tag="base")
                    for dt in range(NDT):
                        nc.tensor.matmul(
                            base_ps[:], lhsT=wtin3_t[:, dt, bk, :],
                            rhs=xt[:, dt, :],
                            start=(dt == 0), stop=(dt == NDT - 1))
                    nc.scalar.copy(out=W3[:, bk, :], in_=base_ps[:])
                base8_ps = ps_base.tile([CD, NC_COLS], f32, tag="base8")
                for dt in range(NDT):
                    nc.tensor.matmul(
                        base8_ps[:], lhsT=wtin8_t[:, dt, :], rhs=xt[:, dt, :],
                        start=(dt == 0), stop=(dt == NDT - 1))
                base8 = smallp.tile([CD, NC_COLS], f32, tag="base8sb")
                nc.scalar.copy(out=base8[:], in_=base8_ps[:])

                for i in range(NQ):
                    bq, qq = i // 4, i % 4
                    # ---- enc_i: accumulate block cross/self terms ----
                    nblk_rd = min(bq + 1, NBLK)
                    parts = nblk_rd + (1 if i == 8 else 0) + \
                        (1 if with_bias else 0)
                    enc_ps = ps_enc.tile([CD, NC_COLS], f32, tag="enc")
                    pi = 0
                    for bk in range(nblk_rd):
                        pi += 1
                        nc.tensor.matmul(
                            enc_ps[:],
                            lhsT=gal3_t[:, bk, CD * i:CD * (i + 1)],
                            rhs=W3[:, bk, :],
                            start=(pi == 1), stop=(pi == parts))
                    if i == 8:
                        pi += 1
                        nc.tensor.matmul(
                            enc_ps[:], lhsT=ident[0:CD, 0:CD], rhs=base8[:],
                            start=False, stop=(pi == parts))
                    if with_bias:
                        pi += 1
                        nc.tensor.matmul(
                            enc_ps[:], lhsT=cbias_t[:, CD * i:CD * (i + 1)],
                            rhs=ones_t[:], start=False, stop=True)
                    enc = encp.tile([CD, NC_COLS], f32, tag="enc_sb")
                    nc.scalar.copy(out=enc[:], in_=enc_ps[:])

                    zq_ps = ps_zq.tile([CD, NC_COLS], f32, tag="zq")
                    for st in range(NST):
                        # ---- scoresT [128, CS] = enc_subtile^T @ cbn_i^T ----
                        sc_ps = ps_sc.tile([128, CS], f32, tag="sc")
                        for h in range(2):
                            nc.tensor.matmul(
                                sc_ps[:, h * 512:(h + 1) * 512],
                                lhsT=enc[:, st * 128:(st + 1) * 128],
                                rhs=cbnt_t[:, i * CS + h * 512:
                                           i * CS + (h + 1) * 512],
                                start=True, stop=True)
                        sc = scp.tile([128, CS], f32, tag="sc_sb")
                        nc.scalar.copy(out=sc[:], in_=sc_ps[:])
                        # ---- top-1 via max8 + max_index ----
                        m8 = smallp.tile([128, 8], f32, tag="m8")
                        nc.vector.max(out=m8[:], in_=sc[:])
                        col = ((i * NCHUNK + c) * NST + st) * 8
                        nc.vector.max_index(
                            out=idxall[:, col:col + 8], in_max=m8[:],
                            in_values=sc[:])
                        # ---- gather zq rows ----
                        zqT = smallp.tile([128, CD], f32, tag="zqT")
                        nc.gpsimd.indirect_dma_start(
                            out=zqT[:], out_offset=None,
                            in_=gtab_d[i][:],
                            in_offset=bass.IndirectOffsetOnAxis(
                                ap=idxall[:, col:col + 1], axis=0),
                        )
                        # ---- transpose into zq psum (partition 0) ----
                        nc.tensor.transpose(
                            out=zq_ps[:, st * 128:(st + 1) * 128],
                            in_=zqT[:], identity=ident[:])
                    zq_sb = smallp.tile([CD, NC_COLS], f32, tag="zq_sb")
                    nc.scalar.copy(out=zq_sb[:], in_=zq_ps[:])
                    # ---- commit zq into workspace quadrant ----
                    if i < 8:
                        qrows = slice(32 * qq, 32 * qq + CD)
                        zqq_ps = ps_zqq.tile([128, NC_COLS], f32, tag="zqq")
                        nc.tensor.matmul(
                            zqq_ps[qrows, :], lhsT=ident[0:CD, 0:CD],
                            rhs=zq_sb[:], start=True, stop=True,
                            tile_position=(0, 32 * qq))
                        nc.scalar.copy(out=W3[qrows, bq, :],
                                       in_=zqq_ps[qrows, :])
                    else:
                        zq8 = zq_sb
                    # ---- loss partial: sum((enc - zq)^2) ----
                    diff = smallp.tile([CD, NC_COLS], f32, tag="diff")
                    nc.vector.tensor_sub(out=diff[:], in0=enc[:],
                                         in1=zq_sb[:])
                    sqs = smallp.tile([CD, NC_COLS], f32, tag="sqs")
                    nc.scalar.activation(
                        out=sqs[:], in_=diff[:],
                        func=mybir.ActivationFunctionType.Square,
                        accum_out=loss_sb[:, c * NQ + i: c * NQ + i + 1])

                # ---- z = OutW_stack @ zq_all ----
                for dt in range(NDT):
                    nzp = NBLK + 1 + (1 if with_bias else 0)
                    z_ps = ps_z.tile([128, NC_COLS], f32, tag="zps")
                    for bk in range(NBLK):
                        nc.tensor.matmul(
                            z_ps[:],
                            lhsT=outwt3_t[:, bk, dt * 128:(dt + 1) * 128],
                            rhs=W3[:, bk, :],
                            start=(bk == 0), stop=False)
                    nc.tensor.matmul(
                        z_ps[:], lhsT=outwt8_t[:, dt * 128:(dt + 1) * 128],
                        rhs=zq8[:], start=False, stop=(not with_bias))
                    if with_bias:
                        nc.tensor.matmul(
                            z_ps[:], lhsT=zbias_t[:, dt * 128:(dt + 1) * 128],
                            rhs=ones_t[:], start=False, stop=True)
                    zsb = zp.tile([128, NC_COLS], f32, tag="zsb")
                    nc.scalar.copy(out=zsb[:], in_=z_ps[:])
                    nc.sync.dma_start(
                        out=z_d[dt * 128:(dt + 1) * 128, cols], in_=zsb[:])

            # ---- ship codes + loss ----
            nc.vector.tensor_reduce(
                out=lossred[:], in_=loss_sb[:],
                axis=mybir.AxisListType.X, op=mybir.AluOpType.add)
            nc.sync.dma_start(out=codes_d[:], in_=idxall[:])
            nc.sync.dma_start(out=loss_d[:], in_=lossred[:])

    nc.finalize()
    return nc


def _prep_weights(in_proj_w, in_proj_b, out_proj_w, out_proj_b, codebooks):
    InW = np.asarray(in_proj_w, np.float32)      # [NQ, CD, D]
    inb = np.asarray(in_proj_b, np.float32)      # [NQ, CD]
    OutW = np.asarray(out_proj_w, np.float32)    # [NQ, D, CD]
    outb = np.asarray(out_proj_b, np.float32)    # [NQ, D]
    cb = np.asarray(codebooks, np.float32)       # [NQ, CS, CD]

    cbn = cb / np.maximum(np.linalg.norm(cb, axis=-1, keepdims=True), 1e-12)

    # quadrant-padded input projections: stage j<8 -> block j//4, quadrant j%4
    wtin3 = np.zeros((D, NBLK, 128), np.float32)
    for j in range(8):
        bk, q = j // 4, j % 4
        wtin3[:, bk, 32 * q:32 * q + CD] = InW[j].T
    wtin8 = np.ascontiguousarray(InW[8].T)       # [D, 8]

    # gal3[bk][32q+r, 8i+c]: stage j=4bk+q cross/self weights for enc_i
    gal3 = np.zeros((128, NBLK, KW), np.float32)
    cbias = inb.copy()
    for i in range(NQ):
        for j in range(min(i + 1, 8)):
            bk, q = j // 4, j % 4
            if j == i:
                blk = np.eye(CD, dtype=np.float32)
            else:
                blk = -(InW[i] @ OutW[j]).T       # [-Mij^T][k, c]
            gal3[32 * q:32 * q + CD, bk, CD * i:CD * (i + 1)] = blk
        for j in range(i):
            cbias[i] -= InW[i] @ outb[j]
    zbias = outb.sum(0)

    cbnt = np.ascontiguousarray(
        cbn.transpose(2, 0, 1).reshape(CD, NQ * CS))             # [8, 9216]

    outwt3 = np.zeros((128, NBLK, D), np.float32)
    for j in range(8):
        bk, q = j // 4, j % 4
        outwt3[32 * q:32 * q + CD, bk, :] = OutW[j].T
    outwt8 = np.ascontiguousarray(OutW[8].T)     # [8, D]

    with_bias = bool(np.any(cbias) or np.any(zbias))
    w = {"wtin3": wtin3, "wtin8": wtin8, "gal3": gal3, "cbnt": cbnt,
         "outwt3": outwt3, "outwt8": outwt8}
    if with_bias:
        w["cbias"] = np.ascontiguousarray(cbias.reshape(1, KW))
        w["zbias"] = np.ascontiguousarray(zbias.reshape(1, D))
    for i in range(NQ):
        w[f"gtab{i}"] = np.ascontiguousarray(cb[i])
    return w, with_bias


TRACE = False
_LAST_PERF = {}


def kernel(x, in_proj_w, in_proj_b, out_proj_w, out_proj_b, codebooks):
    from concourse.bass_utils import run_bass_kernel_spmd

    x = np.asarray(x, np.float32)
    wmap, with_bias = _prep_weights(
        in_proj_w, in_proj_b, out_proj_w, out_proj_b, codebooks)
    key = ("nc", with_bias)
    if key not in _CACHE:
        _CACHE[key] = _build_program(with_bias)
    nc = _CACHE[key]

    in_maps = []
    for b in range(NCORES):
        m = {"x": np.ascontiguousarray(x[b])}
        m.update(wmap)
        in_maps.append(m)

    res = run_bass_kernel_spmd(
        nc, in_maps, core_ids=list(range(NCORES)), trace=TRACE)
    _LAST_PERF["exec_time_ns"] = res.exec_time_ns
    _LAST_PERF["res"] = res

    z = np.stack([res.results[b]["z"] for b in range(NCORES)], axis=0)

    codes = np.zeros((B, NQ, T), np.int32)
    loss_sum = np.float64(0.0)
    for b in range(NCORES):
        st = res.results[b]["codesst"].reshape(128, NQ, NCHUNK, NST, 8)
        # t = c*512 + st*128 + p
        idx = st[:, :, :, :, 0].transpose(1, 2, 3, 0).reshape(NQ, T)
        codes[b] = idx.astype(np.int32)
        loss_sum += np.float64(res.results[b]["lossp"].sum())

    loss = np.float32(loss_sum / (B * CD * T * NQ))
    return z, codes, loss, loss


# revision 20
# speedup vs baseline: 1.0294x; 1.0294x over previous
"""Trainium2 Bass kernel for DAC-style residual VQ bottleneck (9 stages).

Algorithm restructure (validated vs reference to ~1e-7, 0 code mismatches):
  - Residual chain collapsed: z_e_i = InW_i @ x - sum_{j<i} (InW_i@OutW_j) @ zq_j.
    The K=1024 projections happen once ("base", stacked) and per-stage work is
    tiny K<=128 matmuls against a workspace whose rows go base_j -> zq_j.
  - Workspace is quadrant-padded: stage j<8 lives at partition 32*(j%4) of
    block j//4 so every engine access is quadrant-aligned (HW requirement);
    stage 8 uses a separate [8, .] path. Single writer engine (ACT) keeps
    per-instruction sync-wait counts low (HW limit).
  - argmin(dist) == argmax(enc . cb_normalized); top-1 via DVE max8+max_index,
    codebook row gather via indirect DMA, transposed back on the PE.
  - z = OutW_stacked @ zq_all; commit_loss == cb_loss partials shipped to host.

Sharding: one batch per NeuronCore (B=8), weights replicated.
"""

import numpy as np

B, D, T = 8, 1024, 2048
NQ, CS, CD = 9, 1024, 8
NCORES = 8
NCHUNK = 4          # column chunks per core
NC_COLS = 512       # columns per chunk
NST = 4             # 128-row subtiles per chunk
NDT = 8             # D tiles of 128
KW = NQ * CD        # 72 stacked rows
NBLK = 2            # quadrant-padded blocks (stages 0..7); stage 8 separate

_CACHE = {}


def _build_program(with_bias, f32r_scores=False, f32r_aux=True):
    import concourse.bass as bass
    import concourse.mybir as mybir
    from concourse import bacc
    from concourse.tile import TileContext
    from concourse.masks import make_identity

    f32 = mybir.dt.float32
    u32 = mybir.dt.uint32

    nc = bacc.Bacc()

    # ---- DRAM I/O ----
    x_d = nc.dram_tensor("x", [D, T], f32, kind="ExternalInput")
    wtin3_d = nc.dram_tensor("wtin3", [D, NBLK, 128], f32, kind="ExternalInput")
    wtin8_d = nc.dram_tensor("wtin8", [D, CD], f32, kind="ExternalInput")
    gal3_d = nc.dram_tensor("gal3", [128, NBLK, KW], f32, kind="ExternalInput")
    cbnt_d = nc.dram_tensor(
        "cbnt", [CD, NQ * CS],
        mybir.dt.float32r if f32r_scores else f32, kind="ExternalInput")
    zf = mybir.dt.float32r if f32r_aux else f32
    outwt3_d = nc.dram_tensor("outwt3", [128, NBLK, D], zf,
                              kind="ExternalInput")
    outwt8_d = nc.dram_tensor("outwt8", [CD, D], zf, kind="ExternalInput")
    if with_bias:
        cbias_d = nc.dram_tensor("cbias", [1, KW], f32, kind="ExternalInput")
        zbias_d = nc.dram_tensor("zbias", [1, D], f32, kind="ExternalInput")
    gtab_d = [
        nc.dram_tensor(f"gtab{i}", [CS, CD], f32, kind="ExternalInput")
        for i in range(NQ)
    ]
    z_d = nc.dram_tensor("z", [D, T], f32, kind="ExternalOutput")
    codes_d = nc.dram_tensor("codesst", [128, NQ * NCHUNK * NST * 8], u32,
                             kind="ExternalOutput")
    loss_d = nc.dram_tensor("lossp", [CD, 1], f32, kind="ExternalOutput")

    with TileContext(nc) as tc:
        with (
            tc.tile_pool(name="const", bufs=1) as constp,
            tc.tile_pool(name="xin", bufs=2) as xpool,
            tc.tile_pool(name="work", bufs=2) as wpool,
            tc.tile_pool(name="enc", bufs=3) as encp,
            tc.tile_pool(name="scores", bufs=6) as scp,
            tc.tile_pool(name="small", bufs=8) as smallp,
            tc.tile_pool(name="med", bufs=3) as medp,
            tc.tile_pool(name="zout", bufs=3) as zp,
            tc.tile_pool(name="psA", bufs=1, space="PSUM") as ps_base,
            tc.tile_pool(name="psE", bufs=1, space="PSUM") as ps_enc,
            tc.tile_pool(name="psS", bufs=1, space="PSUM") as ps_sc,
            tc.tile_pool(name="psQ", bufs=1, space="PSUM") as ps_zq,
            tc.tile_pool(name="psQQ", bufs=1, space="PSUM") as ps_zqq,
            tc.tile_pool(name="psZ", bufs=1, space="PSUM") as ps_z,
        ):
            # ---- constants / weights to SBUF ----
            wtin3_t = constp.tile([128, NDT, NBLK, 128], f32, tag="wtin3")
            nc.sync.dma_start(
                out=wtin3_t[:],
                in_=wtin3_d.rearrange("(a p) b m -> p a b m", p=128))
            wtin8_t = constp.tile([128, NDT, CD], f32, tag="wtin8")
            nc.sync.dma_start(
                out=wtin8_t[:],
                in_=wtin8_d.rearrange("(a p) m -> p a m", p=128))
            gal3_t = constp.tile([128, NBLK, KW], f32, tag="gal3")
            nc.sync.dma_start(out=gal3_t[:], in_=gal3_d[:])
            cbnt_t = constp.tile(
                [CD, NQ * CS], mybir.dt.float32r if f32r_scores else f32,
                tag="cbnt")
            nc.sync.dma_start(out=cbnt_t[:], in_=cbnt_d[:])
            outwt3_t = constp.tile([128, NBLK, D], zf, tag="outwt3")
            nc.sync.dma_start(out=outwt3_t[:], in_=outwt3_d[:])
            outwt8_t = constp.tile([CD, D], zf, tag="outwt8")
            nc.sync.dma_start(out=outwt8_t[:], in_=outwt8_d[:])
            if with_bias:
                cbias_t = constp.tile([1, KW], f32, tag="cbias")
                nc.sync.dma_start(out=cbias_t[:], in_=cbias_d[:])
                zbias_t = constp.tile([1, D], f32, tag="zbias")
                nc.sync.dma_start(out=zbias_t[:], in_=zbias_d[:])
                ones_t = constp.tile([1, NC_COLS], f32, tag="ones")
                nc.vector.memset(ones_t[:], 1.0)
            ident = constp.tile([128, 128], f32, tag="ident")
            make_identity(nc, ident[:])
            # PE warm-reads: absorb each weight tensor's DMA wait into the
            # PE vector clock up front (matmuls may carry only 1 sync wait).
            warm_ps = ps_base.tile([1, 16], f32, tag="base")
            warm_srcs = [wtin3_t[:, 0, 0, 0:1], wtin8_t[:, 0, 0:1],
                         gal3_t[:, 0, 0:1], cbnt_t[:, 0:1].bitcast(f32),
                         outwt3_t[:, 0, 0:1].bitcast(f32), outwt8_t[:, 0:1].bitcast(f32), ident[:, 0:1]]
            if with_bias:
                warm_srcs += [cbias_t[:, 0:1], zbias_t[:, 0:1], ones_t[:, 0:1]]
            for k, src in enumerate(warm_srcs):
                nc.tensor.matmul(warm_ps[0:1, k:k + 1], lhsT=src, rhs=src,
                                 start=True, stop=True)
            idxall = constp.tile([128, NQ * NCHUNK * NST * 8], u32, tag="idxall")
            loss_sb = constp.tile([CD, 40], f32, tag="loss")
            nc.vector.memset(loss_sb[:], 0)
            lossred = constp.tile([CD, 1], f32, tag="lossred")

            for c in range(NCHUNK):
                cols = slice(c * NC_COLS, (c + 1) * NC_COLS)
                # ---- load x chunk (single DMA -> single wait lane) ----
                xt = xpool.tile([128, NDT, NC_COLS], f32, tag="x")
                nc.sync.dma_start(
                    out=xt[:],
                    in_=x_d[:, cols].rearrange("(a p) n -> p a n", p=128))
                # ---- base (quadrant-padded blocks + stage-8) ----
                W3 = wpool.tile([128, NBLK, NC_COLS], f32, tag="W3")
                W3r = None
                if f32r_aux:
                    W3r = wpool.tile([128, NBLK, NC_COLS],
                                     mybir.dt.float32r, tag="W3r")
                for bk in range(NBLK):
                    base_ps = ps_base.tile([128, NC_COLS], f32, tag="base")
                    for dt in range(NDT):
                        nc.tensor.matmul(
                            base_ps[:], lhsT=wtin3_t[:, dt, bk, :],
                            rhs=xt[:, dt, :],
                            start=(dt == 0), stop=(dt == NDT - 1))
                    nc.scalar.copy(out=W3[:, bk, :], in_=base_ps[:])
                    if f32r_aux:
                        nc.scalar.copy(out=W3r[:, bk, :], in_=base_ps[:])
                base8_ps = ps_base.tile([CD, NC_COLS], f32, tag="base8")
                for dt in range(NDT):
                    nc.tensor.matmul(
                        base8_ps[:], lhsT=wtin8_t[:, dt, :], rhs=xt[:, dt, :],
                        start=(dt == 0), stop=(dt == NDT - 1))
                base8 = medp.tile([CD, NC_COLS], f32, tag="base8sb")
                nc.scalar.copy(out=base8[:], in_=base8_ps[:])

                for i in range(NQ):
                    bq, qq = i // 4, i % 4
                    # ---- enc_i: accumulate block cross/self terms ----
                    nblk_rd = min(bq + 1, NBLK)
                    parts = nblk_rd + (1 if i == 8 else 0) + \
                        (1 if with_bias else 0)
                    enc_ps = ps_enc.tile([CD, NC_COLS], f32, tag="enc")
                    pi = 0
                    for bk in range(nblk_rd):
                        pi += 1
                        nc.tensor.matmul(
                            enc_ps[:],
                            lhsT=gal3_t[:, bk, CD * i:CD * (i + 1)],
                            rhs=W3[:, bk, :],
                            start=(pi == 1), stop=(pi == parts))
                    if i == 8:
                        pi += 1
                        nc.tensor.matmul(
                            enc_ps[:], lhsT=ident[0:CD, 0:CD], rhs=base8[:],
                            start=False, stop=(pi == parts))
                    if with_bias:
                        pi += 1
                        nc.tensor.matmul(
                            enc_ps[:], lhsT=cbias_t[:, CD * i:CD * (i + 1)],
                            rhs=ones_t[:], start=False, stop=True)
                    enc = encp.tile(
                        [CD, NC_COLS],
                        mybir.dt.float32r if f32r_scores else f32,
                        tag="enc_sb")
                    nc.scalar.copy(out=enc[:], in_=enc_ps[:])

                    zq_ps = ps_zq.tile([CD, NC_COLS], f32, tag="zq")
                    for st in range(NST):
                        # ---- scoresT [128, CS] = enc_subtile^T @ cbn_i^T ----
                        sc_ps = ps_sc.tile([128, CS], f32, tag="sc")
                        for h in range(2):
                            nc.tensor.matmul(
                                sc_ps[:, h * 512:(h + 1) * 512],
                                lhsT=enc[:, st * 128:(st + 1) * 128],
                                rhs=cbnt_t[:, i * CS + h * 512:
                                           i * CS + (h + 1) * 512],
                                start=True, stop=True)
                        sc = scp.tile([128, CS], f32, tag="sc_sb")
                        nc.scalar.copy(out=sc[:, 0:512], in_=sc_ps[:, 0:512])
                        nc.scalar.copy(out=sc[:, 512:1024],
                                       in_=sc_ps[:, 512:1024])
                        # ---- top-1 via max8 + max_index ----
                        m8 = smallp.tile([128, 8], f32, tag="m8")
                        nc.vector.max(out=m8[:], in_=sc[:])
                        col = ((i * NCHUNK + c) * NST + st) * 8
                        nc.vector.max_index(
                            out=idxall[:, col:col + 8], in_max=m8[:],
                            in_values=sc[:])
                        # ---- gather zq rows ----
                        zqT = smallp.tile([128, CD], f32, tag="zqT")
                        nc.gpsimd.indirect_dma_start(
                            out=zqT[:], out_offset=None,
                            in_=gtab_d[i][:],
                            in_offset=bass.IndirectOffsetOnAxis(
                                ap=idxall[:, col:col + 1], axis=0),
                        )
                        # ---- transpose into zq psum (partition 0) ----
                        nc.tensor.transpose(
                            out=zq_ps[:, st * 128:(st + 1) * 128],
                            in_=zqT[:], identity=ident[:])
                    zq_sb = medp.tile([CD, NC_COLS], f32, tag="zq_sb")
                    nc.scalar.copy(out=zq_sb[:], in_=zq_ps[:])
                    # ---- commit zq into workspace quadrant ----
                    if i < 8:
                        qrows = slice(32 * qq, 32 * qq + CD)
                        zqq_ps = ps_zqq.tile([128, NC_COLS], f32, tag="zqq")
                        nc.tensor.matmul(
                            zqq_ps[qrows, :], lhsT=ident[0:CD, 0:CD],
                            rhs=zq_sb[:], start=True, stop=True,
                            tile_position=(0, 32 * qq))
                        nc.scalar.copy(out=W3[qrows, bq, :],
                                       in_=zqq_ps[qrows, :])
                        if f32r_aux:
                            nc.scalar.copy(out=W3r[qrows, bq, :],
                                           in_=zqq_ps[qrows, :])
                    else:
                        zq8 = zq_sb
                        if f32r_aux:
                            zq8r = medp.tile(
                                [CD, NC_COLS], mybir.dt.float32r, tag="zq8r")
                            nc.scalar.copy(out=zq8r[:], in_=zq_ps[:])
                    # ---- loss partial: sum((enc - zq)^2) ----
                    diff = medp.tile([CD, NC_COLS], f32, tag="diff")
                    nc.vector.tensor_sub(out=diff[:], in0=enc[:],
                                         in1=zq_sb[:])
                    sqs = medp.tile([CD, NC_COLS], f32, tag="sqs")
                    nc.scalar.activation(
                        out=sqs[:], in_=diff[:],
                        func=mybir.ActivationFunctionType.Square,
                        accum_out=loss_sb[:, c * NQ + i: c * NQ + i + 1])

                # ---- z = OutW_stack @ zq_all ----
                for dt in range(NDT):
                    nzp = NBLK + 1 + (1 if with_bias else 0)
                    z_ps = ps_z.tile([128, NC_COLS], f32, tag="zps")
                    _zw3 = W3r if f32r_aux else W3
                    _zq8 = zq8r if f32r_aux else zq8
                    for bk in range(NBLK):
                        nc.tensor.matmul(
                            z_ps[:],
                            lhsT=outwt3_t[:, bk, dt * 128:(dt + 1) * 128],
                            rhs=_zw3[:, bk, :],
                            start=(bk == 0), stop=False)
                    nc.tensor.matmul(
                        z_ps[:], lhsT=outwt8_t[:, dt * 128:(dt + 1) * 128],
                        rhs=_zq8[:], start=False, stop=(not with_bias))
                    if with_bias:
                        nc.tensor.matmul(
                            z_ps[:], lhsT=zbias_t[:, dt * 128:(dt + 1) * 128],
                            rhs=ones_t[:], start=False, stop=True)
                    zsb = zp.tile([128, NC_COLS], f32, tag="zsb")
                    nc.scalar.copy(out=zsb[:], in_=z_ps[:])
                    nc.sync.dma_start(
                        out=z_d[dt * 128:(dt + 1) * 128, cols], in_=zsb[:])

            # ---- ship codes + loss ----
            nc.vector.tensor_reduce(
                out=lossred[:], in_=loss_sb[:],
                axis=mybir.AxisListType.X, op=mybir.AluOpType.add)
            nc.sync.dma_start(out=codes_d[:], in_=idxall[:])
            nc.sync.dma_start(out=loss_d[:], in_=lossred[:])

    nc.finalize()
    return nc


def _prep_weights(in_proj_w, in_proj_b, out_proj_w, out_proj_b, codebooks):
    InW = np.asarray(in_proj_w, np.float32)      # [NQ, CD, D]
    inb = np.asarray(in_proj_b, np.float32)      # [NQ, CD]
    OutW = np.asarray(out_proj_w, np.float32)    # [NQ, D, CD]
    outb = np.asarray(out_proj_b, np.float32)    # [NQ, D]
    cb = np.asarray(codebooks, np.float32)       # [NQ, CS, CD]

    cbn = cb / np.maximum(np.linalg.norm(cb, axis=-1, keepdims=True), 1e-12)

    # quadrant-padded input projections: stage j<8 -> block j//4, quadrant j%4
    wtin3 = np.zeros((D, NBLK, 128), np.float32)
    for j in range(8):
        bk, q = j // 4, j % 4
        wtin3[:, bk, 32 * q:32 * q + CD] = InW[j].T
    wtin8 = np.ascontiguousarray(InW[8].T)       # [D, 8]

    # gal3[bk][32q+r, 8i+c]: stage j=4bk+q cross/self weights for enc_i
    gal3 = np.zeros((128, NBLK, KW), np.float32)
    cbias = inb.copy()
    for i in range(NQ):
        for j in range(min(i + 1, 8)):
            bk, q = j // 4, j % 4
            if j == i:
                blk = np.eye(CD, dtype=np.float32)
            else:
                blk = -(InW[i] @ OutW[j]).T       # [-Mij^T][k, c]
            gal3[32 * q:32 * q + CD, bk, CD * i:CD * (i + 1)] = blk
        for j in range(i):
            cbias[i] -= InW[i] @ outb[j]
    zbias = outb.sum(0)

    cbnt = np.ascontiguousarray(
        cbn.transpose(2, 0, 1).reshape(CD, NQ * CS))             # [8, 9216]

    outwt3 = np.zeros((128, NBLK, D), np.float32)
    for j in range(8):
        bk, q = j // 4, j % 4
        outwt3[32 * q:32 * q + CD, bk, :] = OutW[j].T
    outwt8 = np.ascontiguousarray(OutW[8].T)     # [8, D]

    with_bias = bool(np.any(cbias) or np.any(zbias))
    w = {"wtin3": wtin3, "wtin8": wtin8, "gal3": gal3, "cbnt": cbnt,
         "outwt3": outwt3, "outwt8": outwt8}
    if with_bias:
        w["cbias"] = np.ascontiguousarray(cbias.reshape(1, KW))
        w["zbias"] = np.ascontiguousarray(zbias.reshape(1, D))
    for i in range(NQ):
        w[f"gtab{i}"] = np.ascontiguousarray(cb[i])
    return w, with_bias


TRACE = False
F32R_SCORES = False
F32R_AUX = True
_LAST_PERF = {}


def kernel(x, in_proj_w, in_proj_b, out_proj_w, out_proj_b, codebooks):
    from concourse.bass_utils import run_bass_kernel_spmd

    x = np.asarray(x, np.float32)
    wmap, with_bias = _prep_weights(
        in_proj_w, in_proj_b, out_proj_w, out_proj_b, codebooks)
    key = ("nc", with_bias, F32R_SCORES, F32R_AUX)
    if key not in _CACHE:
        _CACHE[key] = _build_program(with_bias, F32R_SCORES, F32R_AUX)
    nc = _CACHE[key]

    in_maps = []
    for b in range(NCORES):
        m = {"x": np.ascontiguousarray(x[b])}
        m.update(wmap)
        in_maps.append(m)

    res = run_bass_kernel_spmd(
        nc, in_maps, core_ids=list(range(NCORES)), trace=TRACE)
    _LAST_PERF["exec_time_ns"] = res.exec_time_ns
    _LAST_PERF["res"] = res

    z = np.stack([res.results[b]["z"] for b in range(NCORES)], axis=0)

    codes = np.zeros((B, NQ, T), np.int32)
    loss_sum = np.float64(0.0)
    for b in range(NCORES):
        st = res.results[b]["codesst"].reshape(128, NQ, NCHUNK, NST, 8)
        # t = c*512 + st*128 + p
        idx = st[:, :, :, :, 0].transpose(1, 2, 3, 0).reshape(NQ, T)
        codes[b] = idx.astype(np.int32)
        loss_sum += np.float64(res.results[b]["lossp"].sum())

    loss = np.float32(loss_sum / (B * CD * T * NQ))
    return z, codes, loss, loss


# revision 21
# speedup vs baseline: 1.0368x; 1.0071x over previous
"""Trainium2 Bass kernel for DAC-style residual VQ bottleneck (9 stages).

Algorithm restructure (validated vs reference to ~1e-7, 0 code mismatches):
  - Residual chain collapsed: z_e_i = InW_i @ x - sum_{j<i} (InW_i@OutW_j) @ zq_j.
    The K=1024 projections happen once ("base", stacked) and per-stage work is
    tiny K<=128 matmuls against a workspace whose rows go base_j -> zq_j.
  - Workspace is quadrant-padded: stage j<8 lives at partition 32*(j%4) of
    block j//4 so every engine access is quadrant-aligned (HW requirement);
    stage 8 uses a separate [8, .] path. Single writer engine (ACT) keeps
    per-instruction sync-wait counts low (HW limit).
  - argmin(dist) == argmax(enc . cb_normalized); top-1 via DVE max8+max_index,
    codebook row gather via indirect DMA, transposed back on the PE.
  - z = OutW_stacked @ zq_all; commit_loss == cb_loss partials shipped to host.

Sharding: one batch per NeuronCore (B=8), weights replicated.
"""

import numpy as np

B, D, T = 8, 1024, 2048
NQ, CS, CD = 9, 1024, 8
NCORES = 8
NCHUNK = 4          # column chunks per core
NC_COLS = 512       # columns per chunk
NST = 4             # 128-row subtiles per chunk
NDT = 8             # D tiles of 128
KW = NQ * CD        # 72 stacked rows
NBLK = 2            # quadrant-padded blocks (stages 0..7); stage 8 separate

_CACHE = {}


def _build_program(with_bias, f32r_scores=False, f32r_aux=True):
    import concourse.bass as bass
    import concourse.mybir as mybir
    from concourse import bacc
    from concourse.tile import TileContext
    from concourse.masks import make_identity

    f32 = mybir.dt.float32
    u32 = mybir.dt.uint32

    nc = bacc.Bacc()

    # ---- DRAM I/O ----
    x_d = nc.dram_tensor("x", [D, T], f32, kind="ExternalInput")
    wtin3_d = nc.dram_tensor("wtin3", [D, NBLK, 128], f32, kind="ExternalInput")
    wtin8_d = nc.dram_tensor("wtin8", [D, CD], f32, kind="ExternalInput")
    gal3_d = nc.dram_tensor("gal3", [128, NBLK, KW], f32, kind="ExternalInput")
    cbnt_d = nc.dram_tensor(
        "cbnt", [CD, NQ * CS],
        mybir.dt.float32r if f32r_scores else f32, kind="ExternalInput")
    zf = mybir.dt.float32r if f32r_aux else f32
    outwt3_d = nc.dram_tensor("outwt3", [128, NBLK, D], zf,
                              kind="ExternalInput")
    outwt8_d = nc.dram_tensor("outwt8", [CD, D], zf, kind="ExternalInput")
    if with_bias:
        cbias_d = nc.dram_tensor("cbias", [1, KW], f32, kind="ExternalInput")
        zbias_d = nc.dram_tensor("zbias", [1, D], f32, kind="ExternalInput")
    gtab_d = [
        nc.dram_tensor(f"gtab{i}", [CS, CD], f32, kind="ExternalInput")
        for i in range(NQ)
    ]
    z_d = nc.dram_tensor("z", [D, T], f32, kind="ExternalOutput")
    codes_d = nc.dram_tensor("codesst", [128, NQ * NCHUNK * NST * 8], u32,
                             kind="ExternalOutput")
    loss_d = nc.dram_tensor("lossp", [CD, 1], f32, kind="ExternalOutput")

    with TileContext(nc) as tc:
        with (
            tc.tile_pool(name="const", bufs=1) as constp,
            tc.tile_pool(name="xin", bufs=2) as xpool,
            tc.tile_pool(name="work", bufs=2) as wpool,
            tc.tile_pool(name="enc", bufs=3) as encp,
            tc.tile_pool(name="scores", bufs=6) as scp,
            tc.tile_pool(name="small", bufs=8) as smallp,
            tc.tile_pool(name="med", bufs=3) as medp,
            tc.tile_pool(name="zout", bufs=3) as zp,
            tc.tile_pool(name="psB", bufs=2, space="PSUM") as ps_big,
            tc.tile_pool(name="psE", bufs=1, space="PSUM") as ps_enc,
            tc.tile_pool(name="psS", bufs=3, space="PSUM") as ps_sc,
            tc.tile_pool(name="psQ", bufs=2, space="PSUM") as ps_zq,
        ):
            # ---- constants / weights to SBUF ----
            wtin3_t = constp.tile([128, NDT, NBLK, 128], f32, tag="wtin3")
            nc.sync.dma_start(
                out=wtin3_t[:],
                in_=wtin3_d.rearrange("(a p) b m -> p a b m", p=128))
            wtin8_t = constp.tile([128, NDT, CD], f32, tag="wtin8")
            nc.sync.dma_start(
                out=wtin8_t[:],
                in_=wtin8_d.rearrange("(a p) m -> p a m", p=128))
            gal3_t = constp.tile([128, NBLK, KW], f32, tag="gal3")
            nc.sync.dma_start(out=gal3_t[:], in_=gal3_d[:])
            cbnt_t = constp.tile(
                [CD, NQ * CS], mybir.dt.float32r if f32r_scores else f32,
                tag="cbnt")
            nc.sync.dma_start(out=cbnt_t[:], in_=cbnt_d[:])
            outwt3_t = constp.tile([128, NBLK, D], zf, tag="outwt3")
            nc.sync.dma_start(out=outwt3_t[:], in_=outwt3_d[:])
            outwt8_t = constp.tile([CD, D], zf, tag="outwt8")
            nc.sync.dma_start(out=outwt8_t[:], in_=outwt8_d[:])
            if with_bias:
                cbias_t = constp.tile([1, KW], f32, tag="cbias")
                nc.sync.dma_start(out=cbias_t[:], in_=cbias_d[:])
                zbias_t = constp.tile([1, D], f32, tag="zbias")
                nc.sync.dma_start(out=zbias_t[:], in_=zbias_d[:])
                ones_t = constp.tile([1, NC_COLS], f32, tag="ones")
                nc.vector.memset(ones_t[:], 1.0)
            ident = constp.tile([128, 128], f32, tag="ident")
            make_identity(nc, ident[:])
            # PE warm-reads: absorb each weight tensor's DMA wait into the
            # PE vector clock up front (matmuls may carry only 1 sync wait).
            warm_ps = ps_big.tile([1, 16], f32, tag="bigmm")
            warm_srcs = [wtin3_t[:, 0, 0, 0:1], wtin8_t[:, 0, 0:1],
                         gal3_t[:, 0, 0:1], cbnt_t[:, 0:1].bitcast(f32),
                         outwt3_t[:, 0, 0:1].bitcast(f32), outwt8_t[:, 0:1].bitcast(f32), ident[:, 0:1]]
            if with_bias:
                warm_srcs += [cbias_t[:, 0:1], zbias_t[:, 0:1], ones_t[:, 0:1]]
            for k, src in enumerate(warm_srcs):
                nc.tensor.matmul(warm_ps[0:1, k:k + 1], lhsT=src, rhs=src,
                                 start=True, stop=True)
            idxall = constp.tile([128, NQ * NCHUNK * NST * 8], u32, tag="idxall")
            loss_sb = constp.tile([CD, 40], f32, tag="loss")
            nc.vector.memset(loss_sb[:], 0)
            lossred = constp.tile([CD, 1], f32, tag="lossred")

            for c in range(NCHUNK):
                cols = slice(c * NC_COLS, (c + 1) * NC_COLS)
                # ---- load x chunk (single DMA -> single wait lane) ----
                xt = xpool.tile([128, NDT, NC_COLS], f32, tag="x")
                nc.sync.dma_start(
                    out=xt[:],
                    in_=x_d[:, cols].rearrange("(a p) n -> p a n", p=128))
                # ---- base (quadrant-padded blocks + stage-8) ----
                W3 = wpool.tile([128, NBLK, NC_COLS], f32, tag="W3")
                W3r = None
                if f32r_aux:
                    W3r = wpool.tile([128, NBLK, NC_COLS],
                                     mybir.dt.float32r, tag="W3r")
                for bk in range(NBLK):
                    base_ps = ps_big.tile([128, NC_COLS], f32, tag="bigmm")
                    for dt in range(NDT):
                        nc.tensor.matmul(
                            base_ps[:], lhsT=wtin3_t[:, dt, bk, :],
                            rhs=xt[:, dt, :],
                            start=(dt == 0), stop=(dt == NDT - 1))
                    nc.scalar.copy(out=W3[:, bk, :], in_=base_ps[:])
                    if f32r_aux:
                        nc.scalar.copy(out=W3r[:, bk, :], in_=base_ps[:])
                base8_ps = ps_big.tile([CD, NC_COLS], f32, tag="bigmm")
                for dt in range(NDT):
                    nc.tensor.matmul(
                        base8_ps[:], lhsT=wtin8_t[:, dt, :], rhs=xt[:, dt, :],
                        start=(dt == 0), stop=(dt == NDT - 1))
                base8 = medp.tile([CD, NC_COLS], f32, tag="base8sb")
                nc.scalar.copy(out=base8[:], in_=base8_ps[:])

                for i in range(NQ):
                    bq, qq = i // 4, i % 4
                    # ---- enc_i: accumulate block cross/self terms ----
                    nblk_rd = min(bq + 1, NBLK)
                    parts = nblk_rd + (1 if i == 8 else 0) + \
                        (1 if with_bias else 0)
                    enc_ps = ps_enc.tile([CD, NC_COLS], f32, tag="enc")
                    pi = 0
                    for bk in range(nblk_rd):
                        pi += 1
                        nc.tensor.matmul(
                            enc_ps[:],
                            lhsT=gal3_t[:, bk, CD * i:CD * (i + 1)],
                            rhs=W3[:, bk, :],
                            start=(pi == 1), stop=(pi == parts))
                    if i == 8:
                        pi += 1
                        nc.tensor.matmul(
                            enc_ps[:], lhsT=ident[0:CD, 0:CD], rhs=base8[:],
                            start=False, stop=(pi == parts))
                    if with_bias:
                        pi += 1
                        nc.tensor.matmul(
                            enc_ps[:], lhsT=cbias_t[:, CD * i:CD * (i + 1)],
                            rhs=ones_t[:], start=False, stop=True)
                    enc = encp.tile(
                        [CD, NC_COLS],
                        mybir.dt.float32r if f32r_scores else f32,
                        tag="enc_sb")
                    nc.scalar.copy(out=enc[:], in_=enc_ps[:])

                    zq_ps = ps_zq.tile([CD, NC_COLS], f32, tag="zqx")
                    for st in range(NST):
                        # ---- scoresT [128, CS] = enc_subtile^T @ cbn_i^T ----
                        sc = scp.tile([128, CS], f32, tag="sc_sb")
                        for h in range(2):
                            sc_ps = ps_sc.tile([128, 512], f32, tag="sc")
                            nc.tensor.matmul(
                                sc_ps[:],
                                lhsT=enc[:, st * 128:(st + 1) * 128],
                                rhs=cbnt_t[:, i * CS + h * 512:
                                           i * CS + (h + 1) * 512],
                                start=True, stop=True)
                            nc.scalar.copy(
                                out=sc[:, h * 512:(h + 1) * 512], in_=sc_ps[:])
                        # ---- top-1 via max8 + max_index ----
                        m8 = smallp.tile([128, 8], f32, tag="m8")
                        nc.vector.max(out=m8[:], in_=sc[:])
                        col = ((i * NCHUNK + c) * NST + st) * 8
                        nc.vector.max_index(
                            out=idxall[:, col:col + 8], in_max=m8[:],
                            in_values=sc[:])
                        # ---- gather zq rows ----
                        zqT = smallp.tile([128, CD], f32, tag="zqT")
                        nc.gpsimd.indirect_dma_start(
                            out=zqT[:], out_offset=None,
                            in_=gtab_d[i][:],
                            in_offset=bass.IndirectOffsetOnAxis(
                                ap=idxall[:, col:col + 1], axis=0),
                        )
                        # ---- transpose into zq psum (partition 0) ----
                        nc.tensor.transpose(
                            out=zq_ps[:, st * 128:(st + 1) * 128],
                            in_=zqT[:], identity=ident[:])
                    zq_sb = medp.tile([CD, NC_COLS], f32, tag="zq_sb")
                    nc.scalar.copy(out=zq_sb[:], in_=zq_ps[:])
                    # ---- commit zq into workspace quadrant ----
                    if i < 8:
                        qrows = slice(32 * qq, 32 * qq + CD)
                        zqq_ps = ps_zq.tile([128, NC_COLS], f32, tag="zqx")
                        nc.tensor.matmul(
                            zqq_ps[qrows, :], lhsT=ident[0:CD, 0:CD],
                            rhs=zq_sb[:], start=True, stop=True,
                            tile_position=(0, 32 * qq))
                        nc.scalar.copy(out=W3[qrows, bq, :],
                                       in_=zqq_ps[qrows, :])
                        if f32r_aux:
                            nc.scalar.copy(out=W3r[qrows, bq, :],
                                           in_=zqq_ps[qrows, :])
                    else:
                        zq8 = zq_sb
                        if f32r_aux:
                            zq8r = medp.tile(
                                [CD, NC_COLS], mybir.dt.float32r, tag="zq8r")
                            nc.scalar.copy(out=zq8r[:], in_=zq_ps[:])
                    # ---- loss partial: sum((enc - zq)^2) ----
                    diff = medp.tile([CD, NC_COLS], f32, tag="diff")
                    nc.vector.tensor_sub(out=diff[:], in0=enc[:],
                                         in1=zq_sb[:])
                    sqs = medp.tile([CD, NC_COLS], f32, tag="sqs")
                    nc.scalar.activation(
                        out=sqs[:], in_=diff[:],
                        func=mybir.ActivationFunctionType.Square,
                        accum_out=loss_sb[:, c * NQ + i: c * NQ + i + 1])

                # ---- z = OutW_stack @ zq_all ----
                for dt in range(NDT):
                    nzp = NBLK + 1 + (1 if with_bias else 0)
                    z_ps = ps_big.tile([128, NC_COLS], f32, tag="bigmm")
                    _zw3 = W3r if f32r_aux else W3
                    _zq8 = zq8r if f32r_aux else zq8
                    for bk in range(NBLK):
                        nc.tensor.matmul(
                            z_ps[:],
                            lhsT=outwt3_t[:, bk, dt * 128:(dt + 1) * 128],
                            rhs=_zw3[:, bk, :],
                            start=(bk == 0), stop=False)
                    nc.tensor.matmul(
                        z_ps[:], lhsT=outwt8_t[:, dt * 128:(dt + 1) * 128],
                        rhs=_zq8[:], start=False, stop=(not with_bias))
                    if with_bias:
                        nc.tensor.matmul(
                            z_ps[:], lhsT=zbias_t[:, dt * 128:(dt + 1) * 128],
                            rhs=ones_t[:], start=False, stop=True)
                    zsb = zp.tile([128, NC_COLS], f32, tag="zsb")
                    nc.scalar.copy(out=zsb[:], in_=z_ps[:])
                    nc.sync.dma_start(
                        out=z_d[dt * 128:(dt + 1) * 128, cols], in_=zsb[:])

            # ---- ship codes + loss ----
            nc.vector.tensor_reduce(
                out=lossred[:], in_=loss_sb[:],
                axis=mybir.AxisListType.X, op=mybir.AluOpType.add)
            nc.sync.dma_start(out=codes_d[:], in_=idxall[:])
            nc.sync.dma_start(out=loss_d[:], in_=lossred[:])

    nc.finalize()
    return nc


def _prep_weights(in_proj_w, in_proj_b, out_proj_w, out_proj_b, codebooks):
    InW = np.asarray(in_proj_w, np.float32)      # [NQ, CD, D]
    inb = np.asarray(in_proj_b, np.float32)      # [NQ, CD]
    OutW = np.asarray(out_proj_w, np.float32)    # [NQ, D, CD]
    outb = np.asarray(out_proj_b, np.float32)    # [NQ, D]
    cb = np.asarray(codebooks, np.float32)       # [NQ, CS, CD]

    cbn = cb / np.maximum(np.linalg.norm(cb, axis=-1, keepdims=True), 1e-12)

    # quadrant-padded input projections: stage j<8 -> block j//4, quadrant j%4
    wtin3 = np.zeros((D, NBLK, 128), np.float32)
    for j in range(8):
        bk, q = j // 4, j % 4
        wtin3[:, bk, 32 * q:32 * q + CD] = InW[j].T
    wtin8 = np.ascontiguousarray(InW[8].T)       # [D, 8]

    # gal3[bk][32q+r, 8i+c]: stage j=4bk+q cross/self weights for enc_i
    gal3 = np.zeros((128, NBLK, KW), np.float32)
    cbias = inb.copy()
    for i in range(NQ):
        for j in range(min(i + 1, 8)):
            bk, q = j // 4, j % 4
            if j == i:
                blk = np.eye(CD, dtype=np.float32)
            else:
                blk = -(InW[i] @ OutW[j]).T       # [-Mij^T][k, c]
            gal3[32 * q:32 * q + CD, bk, CD * i:CD * (i + 1)] = blk
        for j in range(i):
            cbias[i] -= InW[i] @ outb[j]
    zbias = outb.sum(0)

    cbnt = np.ascontiguousarray(
        cbn.transpose(2, 0, 1).reshape(CD, NQ * CS))             # [8, 9216]

    outwt3 = np.zeros((128, NBLK, D), np.float32)
    for j in range(8):
        bk, q = j // 4, j % 4
        outwt3[32 * q:32 * q + CD, bk, :] = OutW[j].T
    outwt8 = np.ascontiguousarray(OutW[8].T)     # [8, D]

    with_bias = bool(np.any(cbias) or np.any(zbias))
    w = {"wtin3": wtin3, "wtin8": wtin8, "gal3": gal3, "cbnt": cbnt,
         "outwt3": outwt3, "outwt8": outwt8}
    if with_bias:
        w["cbias"] = np.ascontiguousarray(cbias.reshape(1, KW))
        w["zbias"] = np.ascontiguousarray(zbias.reshape(1, D))
    for i in range(NQ):
        w[f"gtab{i}"] = np.ascontiguousarray(cb[i])
    return w, with_bias


TRACE = False
F32R_SCORES = False
F32R_AUX = True
_LAST_PERF = {}


def kernel(x, in_proj_w, in_proj_b, out_proj_w, out_proj_b, codebooks):
    from concourse.bass_utils import run_bass_kernel_spmd

    x = np.asarray(x, np.float32)
    wmap, with_bias = _prep_weights(
        in_proj_w, in_proj_b, out_proj_w, out_proj_b, codebooks)
    key = ("nc", with_bias, F32R_SCORES, F32R_AUX)
    if key not in _CACHE:
        _CACHE[key] = _build_program(with_bias, F32R_SCORES, F32R_AUX)
    nc = _CACHE[key]

    in_maps = []
    for b in range(NCORES):
        m = {"x": np.ascontiguousarray(x[b])}
        m.update(wmap)
        in_maps.append(m)

    res = run_bass_kernel_spmd(
        nc, in_maps, core_ids=list(range(NCORES)), trace=TRACE)
    _LAST_PERF["exec_time_ns"] = res.exec_time_ns
    _LAST_PERF["res"] = res

    z = np.stack([res.results[b]["z"] for b in range(NCORES)], axis=0)

    codes = np.zeros((B, NQ, T), np.int32)
    loss_sum = np.float64(0.0)
    for b in range(NCORES):
        st = res.results[b]["codesst"].reshape(128, NQ, NCHUNK, NST, 8)
        # t = c*512 + st*128 + p
        idx = st[:, :, :, :, 0].transpose(1, 2, 3, 0).reshape(NQ, T)
        codes[b] = idx.astype(np.int32)
        loss_sum += np.float64(res.results[b]["lossp"].sum())

    loss = np.float32(loss_sum / (B * CD * T * NQ))
    return z, codes, loss, loss


# revision 22
# speedup vs baseline: 1.0487x; 1.0115x over previous
"""Trainium2 Bass kernel for DAC-style residual VQ bottleneck (9 stages).

Algorithm restructure (validated vs reference to ~1e-7, 0 code mismatches):
  - Residual chain collapsed: z_e_i = InW_i @ x - sum_{j<i} (InW_i@OutW_j) @ zq_j.
    The K=1024 projections happen once ("base", stacked) and per-stage work is
    tiny K<=128 matmuls against a workspace whose rows go base_j -> zq_j.
  - Workspace is quadrant-padded: stage j<8 lives at partition 32*(j%4) of
    block j//4 so every engine access is quadrant-aligned (HW requirement);
    stage 8 uses a separate [8, .] path. Single writer engine (ACT) keeps
    per-instruction sync-wait counts low (HW limit).
  - argmin(dist) == argmax(enc . cb_normalized); top-1 via DVE max8+max_index,
    codebook row gather via indirect DMA, transposed back on the PE.
  - z = OutW_stacked @ zq_all; commit_loss == cb_loss partials shipped to host.

Sharding: one batch per NeuronCore (B=8), weights replicated.
"""

import numpy as np

B, D, T = 8, 1024, 2048
NQ, CS, CD = 9, 1024, 8
NCORES = 8
NCHUNK = 4          # column chunks per core
NC_COLS = 512       # columns per chunk
NST = 4             # 128-row subtiles per chunk
NDT = 8             # D tiles of 128
KW = NQ * CD        # 72 stacked rows
NBLK = 2            # quadrant-padded blocks (stages 0..7); stage 8 separate

_CACHE = {}


def _build_program(with_bias, f32r_scores=False, f32r_aux=True):
    import concourse.bass as bass
    import concourse.mybir as mybir
    from concourse import bacc
    from concourse.tile import TileContext
    from concourse.masks import make_identity

    f32 = mybir.dt.float32
    u32 = mybir.dt.uint32

    nc = bacc.Bacc()

    # ---- DRAM I/O ----
    x_d = nc.dram_tensor("x", [D, T], f32, kind="ExternalInput")
    wtin3_d = nc.dram_tensor("wtin3", [D, NBLK, 128], f32, kind="ExternalInput")
    wtin8_d = nc.dram_tensor("wtin8", [D, CD], f32, kind="ExternalInput")
    gal3_d = nc.dram_tensor("gal3", [128, NBLK, KW], f32, kind="ExternalInput")
    cbnt_d = nc.dram_tensor(
        "cbnt", [CD, NQ * CS],
        mybir.dt.float32r if f32r_scores else f32, kind="ExternalInput")
    zf = mybir.dt.float32r if f32r_aux else f32
    outwt3_d = nc.dram_tensor("outwt3", [128, NBLK, D], zf,
                              kind="ExternalInput")
    outwt8_d = nc.dram_tensor("outwt8", [CD, D], zf, kind="ExternalInput")
    if with_bias:
        cbias_d = nc.dram_tensor("cbias", [1, KW], f32, kind="ExternalInput")
        zbias_d = nc.dram_tensor("zbias", [1, D], f32, kind="ExternalInput")
    gtab_d = [
        nc.dram_tensor(f"gtab{i}", [CS, CD], f32, kind="ExternalInput")
        for i in range(NQ)
    ]
    z_d = nc.dram_tensor("z", [D, T], f32, kind="ExternalOutput")
    codes_d = nc.dram_tensor("codesst", [128, NQ * NCHUNK * NST * 8], u32,
                             kind="ExternalOutput")
    loss_d = nc.dram_tensor("lossp", [CD, 1], f32, kind="ExternalOutput")

    with TileContext(nc) as tc:
        with (
            tc.tile_pool(name="const", bufs=1) as constp,
            tc.tile_pool(name="xin", bufs=2) as xpool,
            tc.tile_pool(name="work", bufs=2) as wpool,
            tc.tile_pool(name="enc", bufs=3) as encp,
            tc.tile_pool(name="scores", bufs=6) as scp,
            tc.tile_pool(name="small", bufs=8) as smallp,
            tc.tile_pool(name="med", bufs=3) as medp,
            tc.tile_pool(name="zout", bufs=3) as zp,
            tc.tile_pool(name="psB", bufs=2, space="PSUM") as ps_big,
            tc.tile_pool(name="psE", bufs=1, space="PSUM") as ps_enc,
            tc.tile_pool(name="psS", bufs=3, space="PSUM") as ps_sc,
            tc.tile_pool(name="psQ", bufs=2, space="PSUM") as ps_zq,
        ):
            # ---- constants / weights to SBUF ----
            wtin3_t = constp.tile([128, NDT, NBLK, 128], f32, tag="wtin3")
            nc.sync.dma_start(
                out=wtin3_t[:],
                in_=wtin3_d.rearrange("(a p) b m -> p a b m", p=128))
            wtin8_t = constp.tile([128, NDT, CD], f32, tag="wtin8")
            nc.sync.dma_start(
                out=wtin8_t[:],
                in_=wtin8_d.rearrange("(a p) m -> p a m", p=128))
            gal3_t = constp.tile([128, NBLK, KW], f32, tag="gal3")
            nc.sync.dma_start(out=gal3_t[:], in_=gal3_d[:])
            cbnt_t = constp.tile(
                [CD, NQ * CS], mybir.dt.float32r if f32r_scores else f32,
                tag="cbnt")
            nc.sync.dma_start(out=cbnt_t[:], in_=cbnt_d[:])
            outwt3_t = constp.tile([128, NBLK, D], zf, tag="outwt3")
            nc.sync.dma_start(out=outwt3_t[:], in_=outwt3_d[:])
            outwt8_t = constp.tile([CD, D], zf, tag="outwt8")
            nc.sync.dma_start(out=outwt8_t[:], in_=outwt8_d[:])
            if with_bias:
                cbias_t = constp.tile([1, KW], f32, tag="cbias")
                nc.sync.dma_start(out=cbias_t[:], in_=cbias_d[:])
                zbias_t = constp.tile([1, D], f32, tag="zbias")
                nc.sync.dma_start(out=zbias_t[:], in_=zbias_d[:])
                ones_t = constp.tile([1, NC_COLS], f32, tag="ones")
                nc.vector.memset(ones_t[:], 1.0)
            ident = constp.tile([128, 128], f32, tag="ident")
            make_identity(nc, ident[:])
            # PE warm-reads: absorb each weight tensor's DMA wait into the
            # PE vector clock up front (matmuls may carry only 1 sync wait).
            warm_ps = ps_big.tile([1, 16], f32, tag="bigmm")
            warm_srcs = [wtin3_t[:, 0, 0, 0:1], wtin8_t[:, 0, 0:1],
                         gal3_t[:, 0, 0:1], cbnt_t[:, 0:1].bitcast(f32),
                         outwt3_t[:, 0, 0:1].bitcast(f32), outwt8_t[:, 0:1].bitcast(f32), ident[:, 0:1]]
            if with_bias:
                warm_srcs += [cbias_t[:, 0:1], zbias_t[:, 0:1], ones_t[:, 0:1]]
            for k, src in enumerate(warm_srcs):
                nc.tensor.matmul(warm_ps[0:1, k:k + 1], lhsT=src, rhs=src,
                                 start=True, stop=True)
            idxall = constp.tile([128, NQ * NCHUNK * NST * 8], u32, tag="idxall")
            loss_sb = constp.tile([CD, 40], f32, tag="loss")
            nc.vector.memset(loss_sb[:], 0)
            lossred = constp.tile([CD, 1], f32, tag="lossred")

            for c in range(NCHUNK):
                cols = slice(c * NC_COLS, (c + 1) * NC_COLS)
                # ---- load x chunk (single DMA -> single wait lane) ----
                xt = xpool.tile([128, NDT, NC_COLS], f32, tag="x")
                nc.sync.dma_start(
                    out=xt[:],
                    in_=x_d[:, cols].rearrange("(a p) n -> p a n", p=128))
                # ---- base (quadrant-padded blocks + stage-8) ----
                W3 = wpool.tile([128, NBLK, NC_COLS], f32, tag="W3")
                W3r = None
                if f32r_aux:
                    W3r = wpool.tile([128, NBLK, NC_COLS],
                                     mybir.dt.float32r, tag="W3r")
                for bk in range(NBLK):
                    base_ps = ps_big.tile([128, NC_COLS], f32, tag="bigmm")
                    for dt in range(NDT):
                        nc.tensor.matmul(
                            base_ps[:], lhsT=wtin3_t[:, dt, bk, :],
                            rhs=xt[:, dt, :],
                            start=(dt == 0), stop=(dt == NDT - 1))
                    nc.scalar.copy(out=W3[:, bk, :], in_=base_ps[:])
                base8_ps = ps_big.tile([CD, NC_COLS], f32, tag="bigmm")
                for dt in range(NDT):
                    nc.tensor.matmul(
                        base8_ps[:], lhsT=wtin8_t[:, dt, :], rhs=xt[:, dt, :],
                        start=(dt == 0), stop=(dt == NDT - 1))
                base8 = medp.tile([CD, NC_COLS], f32, tag="base8sb")
                nc.scalar.copy(out=base8[:], in_=base8_ps[:])

                for i in range(NQ):
                    bq, qq = i // 4, i % 4
                    # ---- enc_i: accumulate block cross/self terms ----
                    nblk_rd = min(bq + 1, NBLK)
                    parts = nblk_rd + (1 if i == 8 else 0) + \
                        (1 if with_bias else 0)
                    enc_ps = ps_enc.tile([CD, NC_COLS], f32, tag="enc")
                    pi = 0
                    for bk in range(nblk_rd):
                        pi += 1
                        nc.tensor.matmul(
                            enc_ps[:],
                            lhsT=gal3_t[:, bk, CD * i:CD * (i + 1)],
                            rhs=W3[:, bk, :],
                            start=(pi == 1), stop=(pi == parts))
                    if i == 8:
                        pi += 1
                        nc.tensor.matmul(
                            enc_ps[:], lhsT=ident[0:CD, 0:CD], rhs=base8[:],
                            start=False, stop=(pi == parts))
                    if with_bias:
                        pi += 1
                        nc.tensor.matmul(
                            enc_ps[:], lhsT=cbias_t[:, CD * i:CD * (i + 1)],
                            rhs=ones_t[:], start=False, stop=True)
                    enc = encp.tile(
                        [CD, NC_COLS],
                        mybir.dt.float32r if f32r_scores else f32,
                        tag="enc_sb")
                    nc.scalar.copy(out=enc[:], in_=enc_ps[:])

                    zq_ps = ps_zq.tile([CD, NC_COLS], f32, tag="zqx")
                    for st in range(NST):
                        # ---- scoresT [128, CS] = enc_subtile^T @ cbn_i^T ----
                        sc = scp.tile([128, CS], f32, tag="sc_sb")
                        for h in range(2):
                            sc_ps = ps_sc.tile([128, 512], f32, tag="sc")
                            nc.tensor.matmul(
                                sc_ps[:],
                                lhsT=enc[:, st * 128:(st + 1) * 128],
                                rhs=cbnt_t[:, i * CS + h * 512:
                                           i * CS + (h + 1) * 512],
                                start=True, stop=True)
                            nc.scalar.copy(
                                out=sc[:, h * 512:(h + 1) * 512], in_=sc_ps[:])
                        # ---- top-1 via max8 + max_index ----
                        m8 = smallp.tile([128, 8], f32, tag="m8")
                        nc.vector.max(out=m8[:], in_=sc[:])
                        col = ((i * NCHUNK + c) * NST + st) * 8
                        nc.vector.max_index(
                            out=idxall[:, col:col + 8], in_max=m8[:],
                            in_values=sc[:])
                        # ---- gather zq rows ----
                        zqT = smallp.tile([128, CD], f32, tag="zqT")
                        nc.gpsimd.indirect_dma_start(
                            out=zqT[:], out_offset=None,
                            in_=gtab_d[i][:],
                            in_offset=bass.IndirectOffsetOnAxis(
                                ap=idxall[:, col:col + 1], axis=0),
                        )
                        # ---- transpose into zq psum (partition 0) ----
                        nc.tensor.transpose(
                            out=zq_ps[:, st * 128:(st + 1) * 128],
                            in_=zqT[:], identity=ident[:])
                    zq_sb = medp.tile([CD, NC_COLS], f32, tag="zq_sb")
                    nc.scalar.copy(out=zq_sb[:], in_=zq_ps[:])
                    # ---- commit zq into workspace quadrant ----
                    if i < 8:
                        qrows = slice(32 * qq, 32 * qq + CD)
                        zqq_ps = ps_zq.tile([128, NC_COLS], f32, tag="zqx")
                        nc.tensor.matmul(
                            zqq_ps[qrows, :], lhsT=ident[0:CD, 0:CD],
                            rhs=zq_sb[:], start=True, stop=True,
                            tile_position=(0, 32 * qq))
                        nc.scalar.copy(out=W3[qrows, bq, :],
                                       in_=zqq_ps[qrows, :])
                    else:
                        zq8 = zq_sb
                        if f32r_aux:
                            zq8r = medp.tile(
                                [CD, NC_COLS], mybir.dt.float32r, tag="zq8r")
                            nc.scalar.copy(out=zq8r[:], in_=zq_ps[:])
                    # ---- loss partial: sum((enc - zq)^2) ----
                    diff = medp.tile([CD, NC_COLS], f32, tag="diff")
                    nc.vector.tensor_sub(out=diff[:], in0=enc[:],
                                         in1=zq_sb[:])
                    sqs = medp.tile([CD, NC_COLS], f32, tag="sqs")
                    nc.scalar.activation(
                        out=sqs[:], in_=diff[:],
                        func=mybir.ActivationFunctionType.Square,
                        accum_out=loss_sb[:, c * NQ + i: c * NQ + i + 1])

                # ---- z = OutW_stack @ zq_all ----
                if f32r_aux:
                    nc.scalar.copy(out=W3r[:, 0, :], in_=W3[:, 0, :])
                    nc.scalar.copy(out=W3r[:, 1, :], in_=W3[:, 1, :])
                for dt in range(NDT):
                    nzp = NBLK + 1 + (1 if with_bias else 0)
                    z_ps = ps_big.tile([128, NC_COLS], f32, tag="bigmm")
                    _zw3 = W3r if f32r_aux else W3
                    _zq8 = zq8r if f32r_aux else zq8
                    for bk in range(NBLK):
                        nc.tensor.matmul(
                            z_ps[:],
                            lhsT=outwt3_t[:, bk, dt * 128:(dt + 1) * 128],
                            rhs=_zw3[:, bk, :],
                            start=(bk == 0), stop=False)
                    nc.tensor.matmul(
                        z_ps[:], lhsT=outwt8_t[:, dt * 128:(dt + 1) * 128],
                        rhs=_zq8[:], start=False, stop=(not with_bias))
                    if with_bias:
                        nc.tensor.matmul(
                            z_ps[:], lhsT=zbias_t[:, dt * 128:(dt + 1) * 128],
                            rhs=ones_t[:], start=False, stop=True)
                    zsb = zp.tile([128, NC_COLS], f32, tag="zsb")
                    nc.scalar.copy(out=zsb[:], in_=z_ps[:])
                    nc.sync.dma_start(
                        out=z_d[dt * 128:(dt + 1) * 128, cols], in_=zsb[:])

            # ---- ship codes + loss ----
            nc.vector.tensor_reduce(
                out=lossred[:], in_=loss_sb[:],
                axis=mybir.AxisListType.X, op=mybir.AluOpType.add)
            nc.sync.dma_start(out=codes_d[:], in_=idxall[:])
            nc.sync.dma_start(out=loss_d[:], in_=lossred[:])

    nc.finalize()
    return nc


def _prep_weights(in_proj_w, in_proj_b, out_proj_w, out_proj_b, codebooks):
    InW = np.asarray(in_proj_w, np.float32)      # [NQ, CD, D]
    inb = np.asarray(in_proj_b, np.float32)      # [NQ, CD]
    OutW = np.asarray(out_proj_w, np.float32)    # [NQ, D, CD]
    outb = np.asarray(out_proj_b, np.float32)    # [NQ, D]
    cb = np.asarray(codebooks, np.float32)       # [NQ, CS, CD]

    cbn = cb / np.maximum(np.linalg.norm(cb, axis=-1, keepdims=True), 1e-12)

    # quadrant-padded input projections: stage j<8 -> block j//4, quadrant j%4
    wtin3 = np.zeros((D, NBLK, 128), np.float32)
    for j in range(8):
        bk, q = j // 4, j % 4
        wtin3[:, bk, 32 * q:32 * q + CD] = InW[j].T
    wtin8 = np.ascontiguousarray(InW[8].T)       # [D, 8]

    # gal3[bk][32q+r, 8i+c]: stage j=4bk+q cross/self weights for enc_i
    gal3 = np.zeros((128, NBLK, KW), np.float32)
    cbias = inb.copy()
    for i in range(NQ):
        for j in range(min(i + 1, 8)):
            bk, q = j // 4, j % 4
            if j == i:
                blk = np.eye(CD, dtype=np.float32)
            else:
                blk = -(InW[i] @ OutW[j]).T       # [-Mij^T][k, c]
            gal3[32 * q:32 * q + CD, bk, CD * i:CD * (i + 1)] = blk
        for j in range(i):
            cbias[i] -= InW[i] @ outb[j]
    zbias = outb.sum(0)

    cbnt = np.ascontiguousarray(
        cbn.transpose(2, 0, 1).reshape(CD, NQ * CS))             # [8, 9216]

    outwt3 = np.zeros((128, NBLK, D), np.float32)
    for j in range(8):
        bk, q = j // 4, j % 4
        outwt3[32 * q:32 * q + CD, bk, :] = OutW[j].T
    outwt8 = np.ascontiguousarray(OutW[8].T)     # [8, D]

    with_bias = bool(np.any(cbias) or np.any(zbias))
    w = {"wtin3": wtin3, "wtin8": wtin8, "gal3": gal3, "cbnt": cbnt,
         "outwt3": outwt3, "outwt8": outwt8}
    if with_bias:
        w["cbias"] = np.ascontiguousarray(cbias.reshape(1, KW))
        w["zbias"] = np.ascontiguousarray(zbias.reshape(1, D))
    for i in range(NQ):
        w[f"gtab{i}"] = np.ascontiguousarray(cb[i])
    return w, with_bias


TRACE = False
F32R_SCORES = False
F32R_AUX = True
_LAST_PERF = {}


def kernel(x, in_proj_w, in_proj_b, out_proj_w, out_proj_b, codebooks):
    from concourse.bass_utils import run_bass_kernel_spmd

    x = np.asarray(x, np.float32)
    wmap, with_bias = _prep_weights(
        in_proj_w, in_proj_b, out_proj_w, out_proj_b, codebooks)
    key = ("nc", with_bias, F32R_SCORES, F32R_AUX)
    if key not in _CACHE:
        _CACHE[key] = _build_program(with_bias, F32R_SCORES, F32R_AUX)
    nc = _CACHE[key]

    in_maps = []
    for b in range(NCORES):
        m = {"x": np.ascontiguousarray(x[b])}
        m.update(wmap)
        in_maps.append(m)

    res = run_bass_kernel_spmd(
        nc, in_maps, core_ids=list(range(NCORES)), trace=TRACE)
    _LAST_PERF["exec_time_ns"] = res.exec_time_ns
    _LAST_PERF["res"] = res

    z = np.stack([res.results[b]["z"] for b in range(NCORES)], axis=0)

    codes = np.zeros((B, NQ, T), np.int32)
    loss_sum = np.float64(0.0)
    for b in range(NCORES):
        st = res.results[b]["codesst"].reshape(128, NQ, NCHUNK, NST, 8)
        # t = c*512 + st*128 + p
        idx = st[:, :, :, :, 0].transpose(1, 2, 3, 0).reshape(NQ, T)
        codes[b] = idx.astype(np.int32)
        loss_sum += np.float64(res.results[b]["lossp"].sum())

    loss = np.float32(loss_sum / (B * CD * T * NQ))
    return z, codes, loss, loss


# revision 24
# speedup vs baseline: 1.1309x; 1.0784x over previous
"""Trainium2 Bass kernel for DAC-style residual VQ bottleneck (9 stages).

Algorithm restructure (validated vs reference to ~1e-7, 0 code mismatches):
  - Residual chain collapsed: z_e_i = InW_i @ x - sum_{j<i} (InW_i@OutW_j) @ zq_j.
    The K=1024 projections happen once ("base", stacked) and per-stage work is
    tiny K<=128 matmuls against a workspace whose rows go base_j -> zq_j.
  - Workspace is quadrant-padded: stage j<8 lives at partition 32*(j%4) of
    block j//4 so every engine access is quadrant-aligned (HW requirement);
    stage 8 uses a separate [8, .] path. Single writer engine (ACT) keeps
    per-instruction sync-wait counts low (HW limit).
  - argmin(dist) == argmax(enc . cb_normalized); top-1 via DVE max8+max_index,
    codebook row gather via indirect DMA, transposed back on the PE.
  - z = OutW_stacked @ zq_all; commit_loss == cb_loss partials shipped to host.

Sharding: one batch per NeuronCore (B=8), weights replicated.
"""

import numpy as np

B, D, T = 8, 1024, 2048
NQ, CS, CD = 9, 1024, 8
NCORES = 8
NCHUNK = 4          # column chunks per core
NC_COLS = 512       # columns per chunk
NST = 4             # 128-row subtiles per chunk
NDT = 8             # D tiles of 128
KW = NQ * CD        # 72 stacked rows
NBLK = 2            # quadrant-padded blocks (stages 0..7); stage 8 separate

_CACHE = {}


def _build_program(with_bias, f32r_scores=False, f32r_aux=True):
    import concourse.bass as bass
    import concourse.mybir as mybir
    from concourse import bacc
    from concourse.tile import TileContext
    from concourse.masks import make_identity

    f32 = mybir.dt.float32
    u32 = mybir.dt.uint32

    nc = bacc.Bacc()

    # ---- DRAM I/O ----
    x_d = nc.dram_tensor("x", [D, T], f32, kind="ExternalInput")
    wtin3_d = nc.dram_tensor("wtin3", [D, NBLK, 128], f32, kind="ExternalInput")
    wtin8_d = nc.dram_tensor("wtin8", [D, CD], f32, kind="ExternalInput")
    gal3_d = nc.dram_tensor("gal3", [128, NBLK, KW], f32, kind="ExternalInput")
    cbnt_d = nc.dram_tensor(
        "cbnt", [CD, NQ * CS],
        mybir.dt.float32r if f32r_scores else f32, kind="ExternalInput")
    zf = mybir.dt.float32r if f32r_aux else f32
    outwt3_d = nc.dram_tensor("outwt3", [128, NBLK, D], zf,
                              kind="ExternalInput")
    outwt8_d = nc.dram_tensor("outwt8", [CD, D], zf, kind="ExternalInput")
    if with_bias:
        cbias_d = nc.dram_tensor("cbias", [1, KW], f32, kind="ExternalInput")
        zbias_d = nc.dram_tensor("zbias", [1, D], f32, kind="ExternalInput")
    gtab_d = [
        nc.dram_tensor(f"gtab{i}", [CS, CD], f32, kind="ExternalInput")
        for i in range(NQ)
    ]
    z_d = nc.dram_tensor("z", [D, T], f32, kind="ExternalOutput")
    codes_d = nc.dram_tensor("codesst", [128, NQ * NCHUNK * NST * 8], u32,
                             kind="ExternalOutput")
    loss_d = nc.dram_tensor("lossp", [CD, 1], f32, kind="ExternalOutput")

    with TileContext(nc) as tc:
        with (
            tc.tile_pool(name="const", bufs=1) as constp,
            tc.tile_pool(name="xin", bufs=3) as xpool,
            tc.tile_pool(name="work", bufs=3) as wpool,
            tc.tile_pool(name="enc", bufs=4) as encp,
            tc.tile_pool(name="scores", bufs=6) as scp,
            tc.tile_pool(name="small", bufs=8) as smallp,
            tc.tile_pool(name="med", bufs=3) as medp,
            tc.tile_pool(name="zout", bufs=3) as zp,
            tc.tile_pool(name="psB", bufs=2, space="PSUM") as ps_big,
            tc.tile_pool(name="psE", bufs=1, space="PSUM") as ps_enc,
            tc.tile_pool(name="psS", bufs=3, space="PSUM") as ps_sc,
            tc.tile_pool(name="psQ", bufs=2, space="PSUM") as ps_zq,
        ):
            # ---- constants / weights to SBUF ----
            wtin3_t = constp.tile([128, NDT, NBLK, 128], f32, tag="wtin3")
            nc.sync.dma_start(
                out=wtin3_t[:],
                in_=wtin3_d.rearrange("(a p) b m -> p a b m", p=128))
            wtin8_t = constp.tile([128, NDT, CD], f32, tag="wtin8")
            nc.sync.dma_start(
                out=wtin8_t[:],
                in_=wtin8_d.rearrange("(a p) m -> p a m", p=128))
            gal3_t = constp.tile([128, NBLK, KW], f32, tag="gal3")
            nc.sync.dma_start(out=gal3_t[:], in_=gal3_d[:])
            cbnt_t = constp.tile(
                [CD, NQ * CS], mybir.dt.float32r if f32r_scores else f32,
                tag="cbnt")
            nc.sync.dma_start(out=cbnt_t[:], in_=cbnt_d[:])
            outwt3_t = constp.tile([128, NBLK, D], zf, tag="outwt3")
            nc.sync.dma_start(out=outwt3_t[:], in_=outwt3_d[:])
            outwt8_t = constp.tile([CD, D], zf, tag="outwt8")
            nc.sync.dma_start(out=outwt8_t[:], in_=outwt8_d[:])
            if with_bias:
                cbias_t = constp.tile([1, KW], f32, tag="cbias")
                nc.sync.dma_start(out=cbias_t[:], in_=cbias_d[:])
                zbias_t = constp.tile([1, D], f32, tag="zbias")
                nc.sync.dma_start(out=zbias_t[:], in_=zbias_d[:])
                ones_t = constp.tile([1, NC_COLS], f32, tag="ones")
                nc.vector.memset(ones_t[:], 1.0)
            ident = constp.tile([128, 128], f32, tag="ident")
            make_identity(nc, ident[:])
            # PE warm-reads: absorb each weight tensor's DMA wait into the
            # PE vector clock up front (matmuls may carry only 1 sync wait).
            warm_ps = ps_big.tile([1, 16], f32, tag="bigmm")
            warm_srcs = [wtin3_t[:, 0, 0, 0:1], wtin8_t[:, 0, 0:1],
                         gal3_t[:, 0, 0:1], cbnt_t[:, 0:1].bitcast(f32),
                         outwt3_t[:, 0, 0:1].bitcast(f32), outwt8_t[:, 0:1].bitcast(f32), ident[:, 0:1]]
            if with_bias:
                warm_srcs += [cbias_t[:, 0:1], zbias_t[:, 0:1], ones_t[:, 0:1]]
            for k, src in enumerate(warm_srcs):
                nc.tensor.matmul(warm_ps[0:1, k:k + 1], lhsT=src, rhs=src,
                                 start=True, stop=True)
            idxall = constp.tile([128, NQ * NCHUNK * NST * 8], u32, tag="idxall")
            loss_sb = constp.tile([CD, 40], f32, tag="loss")
            nc.vector.memset(loss_sb[:], 0)
            lossred = constp.tile([CD, 1], f32, tag="lossred")

            for c in range(NCHUNK):
                cols = slice(c * NC_COLS, (c + 1) * NC_COLS)
                # ---- load x chunk (single DMA -> single wait lane) ----
                xt = xpool.tile([128, NDT, NC_COLS], f32, tag="x")
                nc.sync.dma_start(
                    out=xt[:],
                    in_=x_d[:, cols].rearrange("(a p) n -> p a n", p=128))
                # ---- base (quadrant-padded blocks + stage-8) ----
                W3 = wpool.tile([128, NBLK, NC_COLS], f32, tag="W3")
                W3r = None
                if f32r_aux:
                    W3r = wpool.tile([128, NBLK, NC_COLS],
                                     mybir.dt.float32r, tag="W3r")
                for bk in range(NBLK):
                    base_ps = ps_big.tile([128, NC_COLS], f32, tag="bigmm")
                    for dt in range(NDT):
                        nc.tensor.matmul(
                            base_ps[:], lhsT=wtin3_t[:, dt, bk, :],
                            rhs=xt[:, dt, :],
                            start=(dt == 0), stop=(dt == NDT - 1))
                    nc.scalar.copy(out=W3[:, bk, :], in_=base_ps[:])
                base8_ps = ps_big.tile([CD, NC_COLS], f32, tag="bigmm")
                for dt in range(NDT):
                    nc.tensor.matmul(
                        base8_ps[:], lhsT=wtin8_t[:, dt, :], rhs=xt[:, dt, :],
                        start=(dt == 0), stop=(dt == NDT - 1))
                base8 = medp.tile([CD, NC_COLS], f32, tag="base8sb")
                nc.scalar.copy(out=base8[:], in_=base8_ps[:])

                for i in range(NQ):
                    bq, qq = i // 4, i % 4
                    # ---- enc_i: accumulate block cross/self terms ----
                    nblk_rd = min(bq + 1, NBLK)
                    parts = nblk_rd + (1 if i == 8 else 0) + \
                        (1 if with_bias else 0)
                    enc_ps = ps_enc.tile([CD, NC_COLS], f32, tag="enc")
                    pi = 0
                    for bk in range(nblk_rd):
                        pi += 1
                        nc.tensor.matmul(
                            enc_ps[:],
                            lhsT=gal3_t[:, bk, CD * i:CD * (i + 1)],
                            rhs=W3[:, bk, :],
                            start=(pi == 1), stop=(pi == parts))
                    if i == 8:
                        pi += 1
                        nc.tensor.matmul(
                            enc_ps[:], lhsT=ident[0:CD, 0:CD], rhs=base8[:],
                            start=False, stop=(pi == parts))
                    if with_bias:
                        pi += 1
                        nc.tensor.matmul(
                            enc_ps[:], lhsT=cbias_t[:, CD * i:CD * (i + 1)],
                            rhs=ones_t[:], start=False, stop=True)
                    enc = encp.tile(
                        [CD, NC_COLS],
                        mybir.dt.float32r if f32r_scores else f32,
                        tag="enc_sb")
                    nc.scalar.copy(out=enc[:], in_=enc_ps[:])

                    zq_ps = ps_zq.tile([CD, NC_COLS], f32, tag="zqx")
                    for st in range(NST):
                        # ---- scoresT [128, CS] = enc_subtile^T @ cbn_i^T ----
                        sc = scp.tile([128, CS], f32, tag="sc_sb")
                        for h in range(2):
                            sc_ps = ps_sc.tile([128, 512], f32, tag="sc")
                            nc.tensor.matmul(
                                sc_ps[:],
                                lhsT=enc[:, st * 128:(st + 1) * 128],
                                rhs=cbnt_t[:, i * CS + h * 512:
                                           i * CS + (h + 1) * 512],
                                start=True, stop=True)
                            nc.scalar.copy(
                                out=sc[:, h * 512:(h + 1) * 512], in_=sc_ps[:])
                        # ---- top-1 via max8 + max_index ----
                        m8 = smallp.tile([128, 8], f32, tag="m8")
                        nc.vector.max(out=m8[:], in_=sc[:])
                        col = ((i * NCHUNK + c) * NST + st) * 8
                        nc.vector.max_index(
                            out=idxall[:, col:col + 8], in_max=m8[:],
                            in_values=sc[:])
                        # ---- gather zq rows ----
                        zqT = smallp.tile([128, CD], f32, tag="zqT")
                        nc.gpsimd.indirect_dma_start(
                            out=zqT[:], out_offset=None,
                            in_=gtab_d[i][:],
                            in_offset=bass.IndirectOffsetOnAxis(
                                ap=idxall[:, col:col + 1], axis=0),
                        )
                        # ---- transpose into zq psum (partition 0) ----
                        nc.tensor.transpose(
                            out=zq_ps[:, st * 128:(st + 1) * 128],
                            in_=zqT[:], identity=ident[:])
                    zq_sb = medp.tile([CD, NC_COLS], f32, tag="zq_sb")
                    nc.scalar.copy(out=zq_sb[:], in_=zq_ps[:])
                    # ---- commit zq into workspace quadrant ----
                    if i < 8:
                        qrows = slice(32 * qq, 32 * qq + CD)
                        zqq_ps = ps_zq.tile([128, NC_COLS], f32, tag="zqx")
                        nc.tensor.matmul(
                            zqq_ps[qrows, :], lhsT=ident[0:CD, 0:CD],
                            rhs=zq_sb[:], start=True, stop=True,
                            tile_position=(0, 32 * qq))
                        nc.scalar.copy(out=W3[qrows, bq, :],
                                       in_=zqq_ps[qrows, :])
                    else:
                        zq8 = zq_sb
                        if f32r_aux:
                            zq8r = medp.tile(
                                [CD, NC_COLS], mybir.dt.float32r, tag="zq8r")
                            nc.scalar.copy(out=zq8r[:], in_=zq_ps[:])
                    # ---- loss partial: sum((enc - zq)^2) ----
                    diff = medp.tile([CD, NC_COLS], f32, tag="diff")
                    nc.gpsimd.tensor_tensor(
                        out=diff[:], in0=enc[:], in1=zq_sb[:],
                        op=mybir.AluOpType.subtract)
                    sqs = medp.tile([CD, NC_COLS], f32, tag="sqs")
                    nc.scalar.activation(
                        out=sqs[:], in_=diff[:],
                        func=mybir.ActivationFunctionType.Square,
                        accum_out=loss_sb[:, c * NQ + i: c * NQ + i + 1])

                # ---- z = OutW_stack @ zq_all ----
                if f32r_aux:
                    nc.scalar.copy(out=W3r[:, 0, :], in_=W3[:, 0, :])
                    nc.scalar.copy(out=W3r[:, 1, :], in_=W3[:, 1, :])
                for dt in range(NDT):
                    nzp = NBLK + 1 + (1 if with_bias else 0)
                    z_ps = ps_big.tile([128, NC_COLS], f32, tag="bigmm")
                    _zw3 = W3r if f32r_aux else W3
                    _zq8 = zq8r if f32r_aux else zq8
                    for bk in range(NBLK):
                        nc.tensor.matmul(
                            z_ps[:],
                            lhsT=outwt3_t[:, bk, dt * 128:(dt + 1) * 128],
                            rhs=_zw3[:, bk, :],
                            start=(bk == 0), stop=False)
                    nc.tensor.matmul(
                        z_ps[:], lhsT=outwt8_t[:, dt * 128:(dt + 1) * 128],
                        rhs=_zq8[:], start=False, stop=(not with_bias))
                    if with_bias:
                        nc.tensor.matmul(
                            z_ps[:], lhsT=zbias_t[:, dt * 128:(dt + 1) * 128],
                            rhs=ones_t[:], start=False, stop=True)
                    zsb = zp.tile([128, NC_COLS], f32, tag="zsb")
                    nc.scalar.copy(out=zsb[:], in_=z_ps[:])
                    nc.sync.dma_start(
                        out=z_d[dt * 128:(dt + 1) * 128, cols], in_=zsb[:])

            # ---- ship codes + loss ----
            nc.vector.tensor_reduce(
                out=lossred[:], in_=loss_sb[:],
                axis=mybir.AxisListType.X, op=mybir.AluOpType.add)
            nc.sync.dma_start(out=codes_d[:], in_=idxall[:])
            nc.sync.dma_start(out=loss_d[:], in_=lossred[:])

    nc.finalize()
    return nc


def _prep_weights(in_proj_w, in_proj_b, out_proj_w, out_proj_b, codebooks):
    InW = np.asarray(in_proj_w, np.float32)      # [NQ, CD, D]
    inb = np.asarray(in_proj_b, np.float32)      # [NQ, CD]
    OutW = np.asarray(out_proj_w, np.float32)    # [NQ, D, CD]
    outb = np.asarray(out_proj_b, np.float32)    # [NQ, D]
    cb = np.asarray(codebooks, np.float32)       # [NQ, CS, CD]

    cbn = cb / np.maximum(np.linalg.norm(cb, axis=-1, keepdims=True), 1e-12)

    # quadrant-padded input projections: stage j<8 -> block j//4, quadrant j%4
    wtin3 = np.zeros((D, NBLK, 128), np.float32)
    for j in range(8):
        bk, q = j // 4, j % 4
        wtin3[:, bk, 32 * q:32 * q + CD] = InW[j].T
    wtin8 = np.ascontiguousarray(InW[8].T)       # [D, 8]

    # gal3[bk][32q+r, 8i+c]: stage j=4bk+q cross/self weights for enc_i
    gal3 = np.zeros((128, NBLK, KW), np.float32)
    cbias = inb.copy()
    for i in range(NQ):
        for j in range(min(i + 1, 8)):
            bk, q = j // 4, j % 4
            if j == i:
                blk = np.eye(CD, dtype=np.float32)
            else:
                blk = -(InW[i] @ OutW[j]).T       # [-Mij^T][k, c]
            gal3[32 * q:32 * q + CD, bk, CD * i:CD * (i + 1)] = blk
        for j in range(i):
            cbias[i] -= InW[i] @ outb[j]
    zbias = outb.sum(0)

    cbnt = np.ascontiguousarray(
        cbn.transpose(2, 0, 1).reshape(CD, NQ * CS))             # [8, 9216]

    outwt3 = np.zeros((128, NBLK, D), np.float32)
    for j in range(8):
        bk, q = j // 4, j % 4
        outwt3[32 * q:32 * q + CD, bk, :] = OutW[j].T
    outwt8 = np.ascontiguousarray(OutW[8].T)     # [8, D]

    with_bias = bool(np.any(cbias) or np.any(zbias))
    w = {"wtin3": wtin3, "wtin8": wtin8, "gal3": gal3, "cbnt": cbnt,
         "outwt3": outwt3, "outwt8": outwt8}
    if with_bias:
        w["cbias"] = np.ascontiguousarray(cbias.reshape(1, KW))
        w["zbias"] = np.ascontiguousarray(zbias.reshape(1, D))
    for i in range(NQ):
        w[f"gtab{i}"] = np.ascontiguousarray(cb[i])
    return w, with_bias


TRACE = False
F32R_SCORES = False
F32R_AUX = True
_LAST_PERF = {}


def kernel(x, in_proj_w, in_proj_b, out_proj_w, out_proj_b, codebooks):
    from concourse.bass_utils import run_bass_kernel_spmd

    x = np.asarray(x, np.float32)
    wmap, with_bias = _prep_weights(
        in_proj_w, in_proj_b, out_proj_w, out_proj_b, codebooks)
    key = ("nc", with_bias, F32R_SCORES, F32R_AUX)
    if key not in _CACHE:
        _CACHE[key] = _build_program(with_bias, F32R_SCORES, F32R_AUX)
    nc = _CACHE[key]

    in_maps = []
    for b in range(NCORES):
        m = {"x": np.ascontiguousarray(x[b])}
        m.update(wmap)
        in_maps.append(m)

    res = run_bass_kernel_spmd(
        nc, in_maps, core_ids=list(range(NCORES)), trace=TRACE)
    _LAST_PERF["exec_time_ns"] = res.exec_time_ns
    _LAST_PERF["res"] = res

    z = np.stack([res.results[b]["z"] for b in range(NCORES)], axis=0)

    codes = np.zeros((B, NQ, T), np.int32)
    loss_sum = np.float64(0.0)
    for b in range(NCORES):
        st = res.results[b]["codesst"].reshape(128, NQ, NCHUNK, NST, 8)
        # t = c*512 + st*128 + p
        idx = st[:, :, :, :, 0].transpose(1, 2, 3, 0).reshape(NQ, T)
        codes[b] = idx.astype(np.int32)
        loss_sum += np.float64(res.results[b]["lossp"].sum())

    loss = np.float32(loss_sum / (B * CD * T * NQ))
    return z, codes, loss, loss
